# revision 1
# baseline (speedup 1.0000x reference)
"""3-layer GraphSAGE (mean aggr + L2 norm) on 8 Trainium2 NeuronCores.

Strategy (graph/data parallel, dst-sharded):
  - Nodes are partitioned into 8 contiguous ranges (12500 per core); each core
    computes all three layers for its destination range.
  - Mean aggregation h_agg = D^-1 A h is computed per core as a stream of
    one-hot scatter matmuls: messages h[src] are gathered from a replicated
    fp16 node table in HBM with dma_gather (int16 indices -> 4 source chunks
    of 25000 rows), then each 128-edge tile is reduced into a 256-wide PSUM
    destination window via PE matmul with a per-tile one-hot matrix S
    (S[e, slot] = 1/deg[dst_e] if slot == dst_e's window slot) built on the
    vector engine from per-edge slot/weight sidebands.
  - Dense part out = mean @ Wl + b + x @ Wr runs on PE per window with
    channel-major rhs; L2 normalization via PE transpose + ACT square/accum.
  - Between layers the 8 per-core node slices are AllGathered (fp16) into a
    replicated full table for the next layer's gathers.
  - The instruction stream is identical on all 8 cores (SPMD); all per-core
    variation lives in input data (indices, slots, weights). Tile counts per
    (window, chunk) group are padded to the max over cores.
"""

import math

import numpy as np

N_NODES = 100000
N_EDGES = 1600000
IN_C, HID_C, OUT_C = 128, 128, 64
EPS = 1e-12

N_CORES = 8
NPC = N_NODES // N_CORES        # nodes per core
WIN = 256                       # psum window width (dst slots)
N_CHUNKS = 4                    # source chunks (int16 index limit)
CHUNK_ROWS = N_NODES // N_CHUNKS
T_CALL = 8                      # tiles per dma_gather call (1024 idxs: HW SWDGE per-call cap)
P = 128

_CACHE = {}
TRACE = False          # set True (e.g. from test.py) to capture an NTFF trace
LAST_RESULT = None     # BassKernelResults of the most recent run


# --------------------------------------------------------------------------
# Host-side preprocessing: edge sort, uniform tile layout, sideband packing
# --------------------------------------------------------------------------

def _wrap_idx(flat: np.ndarray) -> np.ndarray:
    """Pack a flat int16 index list (len % 16 == 0) into the dma_gather
    wrapped layout [16, n/16] replicated to 128 partitions."""
    n = len(flat)
    arr = flat.reshape(n // 16, 16).T.astype(np.int16)
    return np.tile(arr, (8, 1))


def _preprocess(edge_index: np.ndarray):
    src = np.ascontiguousarray(edge_index[0]).astype(np.int64)
    dst = np.ascontiguousarray(edge_index[1]).astype(np.int64)
    deg = np.bincount(dst, minlength=N_NODES)
    winv = (1.0 / np.maximum(deg, 1.0)).astype(np.float32)

    core = dst // NPC
    w_in = (dst % NPC) // WIN
    chunk = src // CHUNK_ROWS
    NW = math.ceil(NPC / WIN)

    # group counts per (core, chunk, window)
    counts = np.zeros((N_CORES, N_CHUNKS, NW), dtype=np.int64)
    np.add.at(counts, (core, chunk, w_in), 1)
    # uniform tile budget per (window, chunk): max over cores
    B = np.ceil(counts.max(axis=0) / P).astype(np.int64)  # [N_CHUNKS, NW]

    # order edges by (core, chunk, window, dst) — chunk-major gather runs
    order = np.lexsort((dst, w_in, chunk, core))
    src_s, dst_s = src[order], dst[order]
    co_s, ch_s, w_s = core[order], chunk[order], w_in[order]

    NT_chunk = B.sum(axis=1)            # tiles per chunk-run  [N_CHUNKS]
    NT_total = int(NT_chunk.sum())      # tiles per core per layer
    pad_edges = NT_total * P            # padded edges per core

    # chunk-run tile offset of (c, w): tiles of chunk c laid out window-major
    cumB = np.zeros((N_CHUNKS, NW + 1), dtype=np.int64)
    cumB[:, 1:] = np.cumsum(B, axis=1)
    chunk_off = np.zeros(N_CHUNKS + 1, dtype=np.int64)
    chunk_off[1:] = np.cumsum(NT_chunk)

    idx_cols = pad_edges // 16
    idx_all = np.zeros((N_CORES, 128, idx_cols), dtype=np.int16)
    slot_all = np.zeros((N_CORES, 128, NT_total), dtype=np.float32)
    wgt_all = np.zeros((N_CORES, 128, NT_total), dtype=np.float32)

    # processing-order global tile index g for (w, c, j):
    # tiles ordered by (w, c, j)
    g_off = np.zeros((NW, N_CHUNKS), dtype=np.int64)
    g = 0
    for w in range(NW):
        for c in range(N_CHUNKS):
            g_off[w, c] = g
            g += int(B[c, w])
    assert g == NT_total

    for k in range(N_CORES):
        sel = co_s == k
        sk, dk, ck, wk = src_s[sel], dst_s[sel], ch_s[sel], w_s[sel]
        # per-(c,w) boundaries within this core's slice (sorted by c, w)
        cnt = np.zeros((N_CHUNKS, NW), dtype=np.int64)
        np.add.at(cnt, (ck, wk), 1)
        starts = np.zeros((N_CHUNKS, NW), dtype=np.int64)
        flat_sizes = cnt.reshape(-1)
        flat_starts = np.zeros_like(flat_sizes)
        flat_starts[1:] = np.cumsum(flat_sizes)[:-1]
        starts = flat_starts.reshape(N_CHUNKS, NW)

        idx_pad = np.zeros(pad_edges, dtype=np.int16)
        slot_pad = np.zeros(pad_edges, dtype=np.float32)
        wgt_pad = np.zeros(pad_edges, dtype=np.float32)
        for c in range(N_CHUNKS):
            for w in range(NW):
                n = int(cnt[c, w])
                bt = int(B[c, w])
                if bt == 0:
                    assert n == 0
                    continue
                s0 = int(starts[c, w])
                e_src = sk[s0 : s0 + n]
                e_dst = dk[s0 : s0 + n]
                p0 = (chunk_off[c] + cumB[c, w]) * P
                idx_pad[p0 : p0 + n] = (e_src - c * CHUNK_ROWS).astype(np.int16)
                slot_pad[p0 : p0 + n] = (e_dst - k * NPC - w * WIN).astype(np.float32)
                wgt_pad[p0 : p0 + n] = winv[e_dst].astype(np.float32)
                # pads: idx 0 (valid row), wgt 0, slot 0

        idx_all[k] = _wrap_idx(idx_pad)
        # tile sidebands in processing order (w, c, j)
        sp = slot_pad.reshape(NT_total, P)  # chunk-run order tiles
        wp = wgt_pad.reshape(NT_total, P)
        for w in range(NW):
            for c in range(N_CHUNKS):
                bt = int(B[c, w])
                if bt == 0:
                    continue
                pos0 = chunk_off[c] + cumB[c, w]
                gg = g_off[w, c]
                slot_all[k, :, gg : gg + bt] = sp[pos0 : pos0 + bt].T
                wgt_all[k, :, gg : gg + bt] = wp[pos0 : pos0 + bt].T

    # gather call layout per chunk: blocks of T_CALL tiles
    calls = []  # list of (chunk, tile_start_in_chunkrun, ntiles)
    for c in range(N_CHUNKS):
        t = 0
        while t < NT_chunk[c]:
            nt = int(min(T_CALL, NT_chunk[c] - t))
            calls.append((c, t, nt))
            t += nt

    struct = {
        "NW": NW,
        "B": B,
        "cumB": cumB,
        "chunk_off": chunk_off,
        "g_off": g_off,
        "NT_total": NT_total,
        "idx_cols": idx_cols,
        "calls": calls,
    }
    return struct, idx_all, slot_all, wgt_all


# --------------------------------------------------------------------------
# Device program
# --------------------------------------------------------------------------

def _build_program(struct):
    import concourse.bacc as bacc
    import concourse.bass as bass
    import concourse.tile as tile
    from concourse import mybir
    from concourse.masks import make_identity

    fp16 = mybir.dt.float16
    f32 = mybir.dt.float32

    NW = struct["NW"]
    B = struct["B"]
    cumB = struct["cumB"]
    chunk_off = struct["chunk_off"]
    g_off = struct["g_off"]
    NT_total = struct["NT_total"]
    idx_cols = struct["idx_cols"]
    calls = struct["calls"]

    nc = bacc.Bacc("TRN2", num_devices=N_CORES)

    xg = nc.dram_tensor("xg", [N_NODES, IN_C], fp16, kind="ExternalInput")
    xt = nc.dram_tensor("xt", [P, NPC], fp16, kind="ExternalInput")
    idx_t = nc.dram_tensor("idx", [128, idx_cols], mybir.dt.int16, kind="ExternalInput")
    slot_t = nc.dram_tensor("slot", [128, NT_total], f32, kind="ExternalInput")
    wgt_t = nc.dram_tensor("wgt", [128, NT_total], f32, kind="ExternalInput")
    wls, bls, wrs = [], [], []
    dims = [(IN_C, HID_C), (HID_C, HID_C), (HID_C, OUT_C)]
    for i, (din, dout) in enumerate(dims):
        wls.append(nc.dram_tensor(f"wl{i}", [din, dout], fp16, kind="ExternalInput"))
        bls.append(nc.dram_tensor(f"bl{i}", [dout, 1], f32, kind="ExternalInput"))
        wrs.append(nc.dram_tensor(f"wr{i}", [din, dout], fp16, kind="ExternalInput"))
    out_t = nc.dram_tensor("out", [NPC, OUT_C], f32, kind="ExternalOutput")

    # inter-layer buffers
    cc_in = [
        nc.dram_tensor(f"cc{i}_in", [NPC, HID_C], fp16, kind="Internal")
        for i in range(2)
    ]
    h_full = [
        nc.dram_tensor(
            f"h{i}_full", [N_NODES, HID_C], fp16, kind="Internal", addr_space="Shared"
        )
        for i in range(2)
    ]
    h_t = [
        nc.dram_tensor(f"h{i}t", [P, NPC], fp16, kind="Internal") for i in range(2)
    ]

    rg = [list(range(N_CORES))]

    with tile.TileContext(nc) as tc:
        with (
            tc.tile_pool(name="const", bufs=1) as cpool,
            tc.tile_pool(name="msg", bufs=2) as mpool,
            tc.tile_pool(name="work", bufs=3) as pool,
            tc.tile_pool(name="spool", bufs=4) as spool,
            tc.tile_pool(name="psum", bufs=2, space="PSUM") as ppool,
        ):
            # constants
            iota_i = cpool.tile([128, WIN], mybir.dt.int32)
            nc.gpsimd.iota(iota_i[:], pattern=[[1, WIN]], base=0, channel_multiplier=0)
            iota_f = cpool.tile([128, WIN], fp16)
            nc.vector.tensor_copy(iota_f[:], iota_i[:])
            ident32 = cpool.tile([128, 128], f32)
            make_identity(nc, ident32[:])
            ident16 = cpool.tile([128, 128], fp16)
            nc.vector.tensor_copy(ident16[:], ident32[:])

            idx_sb = cpool.tile([128, idx_cols], mybir.dt.int16)
            nc.sync.dma_start(idx_sb[:], idx_t[:])
            slot_sb = cpool.tile([128, NT_total], f32)
            nc.sync.dma_start(slot_sb[:], slot_t[:])
            wgt_sb = cpool.tile([128, NT_total], f32)
            nc.sync.dma_start(wgt_sb[:], wgt_t[:])

            wl_sb, bl_sb, wr_sb = [], [], []
            for i, (din, dout) in enumerate(dims):
                wl = cpool.tile([din, dout], fp16, tag=f"wl{i}")
                nc.sync.dma_start(wl[:], wls[i][:])
                bl = cpool.tile([dout, 1], f32, tag=f"bl{i}")
                nc.sync.dma_start(bl[:], bls[i][:])
                wr = cpool.tile([din, dout], fp16, tag=f"wr{i}")
                nc.sync.dma_start(wr[:], wrs[i][:])
                wl_sb.append(wl)
                bl_sb.append(bl)
                wr_sb.append(wr)

            for L in range(3):
                table = [xg, h_full[0], h_full[1]][L]
                xtab = [xt, h_t[0], h_t[1]][L]
                co = dims[L][1]

                # gather call stream state
                call_bufs = {}   # call index (in `calls`) -> sbuf tile
                next_call = 0
                covered = [0] * N_CHUNKS  # tiles covered per chunk-run
                call_base = {}
                for k2, (cc2, tt0, _nt) in enumerate(calls):
                    call_base.setdefault(cc2, k2)

                def emit_call(ci):
                    c, t0, nt = calls[ci]
                    buf = mpool.tile([128, T_CALL, 128], fp16, tag=f"g{c}")
                    col0 = (chunk_off[c] + t0) * P // 16
                    ncols = nt * P // 16
                    nc.gpsimd.dma_gather(
                        buf[:, :nt, :],
                        table[c * CHUNK_ROWS : (c + 1) * CHUNK_ROWS, :],
                        idx_sb[:, col0 : col0 + ncols],
                        nt * P,
                        nt * P,
                        128,
                    )
                    return buf

                for w in range(NW):
                    need = [int(cumB[c, w + 1]) for c in range(N_CHUNKS)]
                    while any(covered[c] < need[c] for c in range(N_CHUNKS)):
                        c, t0, nt = calls[next_call]
                        call_bufs[next_call] = emit_call(next_call)
                        covered[c] = t0 + nt
                        next_call += 1

                    wn = min(WIN, NPC - w * WIN)
                    psum = ppool.tile([128, WIN], f32, tag="agg")
                    nc.vector.memset(psum[:], 0.0)
                    ntiles_w = int(sum(B[c, w] for c in range(N_CHUNKS)))
                    done = 0
                    for c in range(N_CHUNKS):
                        bt = int(B[c, w])
                        for j in range(bt):
                            g = int(g_off[w, c]) + j
                            pos = int(cumB[c, w]) + j  # tile pos in chunk-run
                            gci = call_base[c] + pos // T_CALL
                            buf = call_bufs[gci]
                            t_in = pos % T_CALL
                            s_tile = spool.tile([128, WIN], fp16, tag="s")
                            nc.vector.tensor_scalar(
                                out=s_tile[:],
                                in0=iota_f[:],
                                scalar1=slot_sb[:, g : g + 1],
                                scalar2=wgt_sb[:, g : g + 1],
                                op0=mybir.AluOpType.is_equal,
                                op1=mybir.AluOpType.mult,
                            )
                            done += 1
                            nc.tensor.matmul(
                                psum[:],
                                lhsT=buf[:, t_in, :],
                                rhs=s_tile[:],
                                start=False,
                                stop=(done == ntiles_w),
                                skip_group_check=True,
                            )

                    # ---- dense phase for window w ----
                    meanT = pool.tile([128, WIN], fp16, tag="meanT")
                    nc.vector.tensor_copy(meanT[:, :wn], psum[:, :wn])
                    xw = pool.tile([128, WIN], fp16, tag="xw")
                    nc.sync.dma_start(xw[:, :wn], xtab[:, w * WIN : w * WIN + wn])
                    psum2 = ppool.tile([co, WIN], f32, tag="dense")
                    nc.tensor.matmul(
                        psum2[:, :wn], lhsT=wl_sb[L][:], rhs=meanT[:, :wn],
                        start=True, stop=False, skip_group_check=True,
                    )
                    nc.tensor.matmul(
                        psum2[:, :wn], lhsT=wr_sb[L][:], rhs=xw[:, :wn],
                        start=False, stop=True, skip_group_check=True,
                    )
                    dsb = pool.tile([co, WIN], f32, tag="dsb")
                    nc.vector.tensor_scalar(
                        out=dsb[:, :wn], in0=psum2[:, :wn],
                        scalar1=bl_sb[L][:], scalar2=None,
                        op0=mybir.AluOpType.add,
                    )
                    n_sub = math.ceil(wn / 128)
                    for sub in range(n_sub):
                        bs = min(128, wn - sub * 128)
                        n0 = w * WIN + sub * 128
                        psum3 = ppool.tile([128, 128], f32, tag="tp")
                        nc.tensor.transpose(
                            psum3[:bs, :co],
                            dsb[:, sub * 128 : sub * 128 + bs],
                            ident32[:co, :co],
                        )
                        sq = pool.tile([128, 128], f32, tag="sq")
                        ssq = pool.tile([128, 1], f32, tag="ssq")
                        nc.scalar.activation(
                            sq[:bs, :co], psum3[:bs, :co],
                            mybir.ActivationFunctionType.Square,
                            accum_out=ssq[:bs, :],
                        )
                        nrm = pool.tile([128, 1], f32, tag="nrm")
                        nc.scalar.activation(
                            nrm[:bs, :], ssq[:bs, :],
                            mybir.ActivationFunctionType.Sqrt,
                        )
                        nc.vector.tensor_scalar(
                            out=nrm[:bs, :], in0=nrm[:bs, :], scalar1=float(EPS),
                            scalar2=None, op0=mybir.AluOpType.max,
                        )
                        rinv = pool.tile([128, 1], f32, tag="rinv")
                        nc.vector.reciprocal(rinv[:bs, :], nrm[:bs, :])
                        if L < 2:
                            hn = pool.tile([128, 128], fp16, tag="hn")
                            nc.scalar.activation(
                                hn[:bs, :co], psum3[:bs, :co],
                                mybir.ActivationFunctionType.Relu,
                                scale=rinv[:bs, :],
                            )
                            nc.sync.dma_start(cc_in[L][n0 : n0 + bs, :], hn[:bs, :co])
                            psum4 = ppool.tile([128, 128], fp16, tag="tp2")
                            nc.tensor.transpose(
                                psum4[:co, :bs], hn[:bs, :co], ident16[:bs, :bs]
                            )
                            hts = pool.tile([128, 128], fp16, tag="hts")
                            nc.vector.tensor_copy(hts[:co, :bs], psum4[:co, :bs])
                            nc.sync.dma_start(
                                h_t[L][:, n0 : n0 + bs], hts[:co, :bs]
                            )
                        else:
                            hn = pool.tile([128, 64], f32, tag="hnf")
                            nc.vector.tensor_scalar(
                                out=hn[:bs, :co], in0=psum3[:bs, :co],
                                scalar1=rinv[:bs, :], scalar2=None,
                                op0=mybir.AluOpType.mult,
                            )
                            nc.sync.dma_start(out_t[n0 : n0 + bs, :], hn[:bs, :co])

                if L < 2:
                    nc.gpsimd.collective_compute(
                        "AllGather",
                        mybir.AluOpType.bypass,
                        replica_groups=rg,
                        ins=[cc_in[L][:]],
                        outs=[h_full[L][:]],
                    )
    nc.compile()
    return nc


# --------------------------------------------------------------------------
# Entry point
# --------------------------------------------------------------------------

def kernel(**inputs) -> np.ndarray:
    from concourse.bass_utils import run_bass_kernel_spmd

    x = np.asarray(inputs["x"], dtype=np.float32)
    edge_index = np.asarray(inputs["edge_index"])

    struct, idx_all, slot_all, wgt_all = _preprocess(edge_index)

    key = ("prog", struct["NT_total"], struct["idx_cols"], tuple(struct["chunk_off"]))
    if key not in _CACHE:
        _CACHE[key] = _build_program(struct)
    nc = _CACHE[key]

    xg = x.astype(np.float16)
    in_maps = []
    for k in range(N_CORES):
        m = {
            "xg": xg,
            "xt": np.ascontiguousarray(
                x[k * NPC : (k + 1) * NPC, :].T.astype(np.float16)
            ),
            "idx": idx_all[k],
            "slot": slot_all[k],
            "wgt": wgt_all[k],
        }
        for i in range(3):
            m[f"wl{i}"] = np.asarray(inputs[f"Wl{i}"], dtype=np.float16)
            m[f"bl{i}"] = np.asarray(inputs[f"bl{i}"], dtype=np.float32).reshape(-1, 1)
            m[f"wr{i}"] = np.asarray(inputs[f"Wr{i}"], dtype=np.float16)
        in_maps.append(m)

    res = run_bass_kernel_spmd(
        nc, in_maps, core_ids=list(range(N_CORES)), trace=TRACE
    )
    global LAST_RESULT
    LAST_RESULT = res
    out = np.concatenate([res.results[k]["out"] for k in range(N_CORES)], axis=0)
    return out.astype(np.float32)

